# revision 16
# baseline (speedup 1.0000x reference)
"""Ragged-sequence LSTM + linear projection on 8 TRN2 NeuronCores.

Reference semantics (torch pack_padded_sequence style):
  h_seq[b,t] = LSTM cell output for t < len[b], else 0
  y = h_seq @ W_out.T + b_out    (padded rows get exactly b_out)

Key facts exploited:
  * The valid-mask is monotone in t, so state freezing past len[b] is
    unnecessary: post-death state never influences a valid output. We only
    need stored h == 0 for dead steps. We get that for free by adding an
    augmented input row x_aug[13] = (t >= len) with weight -60 on the o-gate
    pre-activation: sigmoid(-60) ~ 0 -> h = 0.
  * Pure data parallel: 64 sequences per core, identical program.

Layout (per core, everything f32):
  h, c stored transposed [50 part, 64 free].  Gates computed transposed in
  PSUM, padded to quadrant boundaries (engine partition accesses must start
  at 0/32/64/96): bank A holds i@0:50, f@64:114; bank B holds g@0:50,
  o@64:114, each [128p x 512f].  Per 8-step block one K=14 matmul per gate
  computes xproj for all 8 steps; per step a K=50 matmul per gate
  accumulates hproj into a 64-col slice.  ACT applies sigmoid/tanh (bias
  folded in), DVE does the cell update, h lands in an SBUF history tile;
  once per block a K=50 matmul projects h->y (13 rows) and the result
  (+b_out) streams to DRAM.
"""

import numpy as np

import concourse.bass as bass
from concourse import bacc
import concourse.mybir as mybir
from concourse import tile
from concourse.bass_utils import run_bass_kernel_spmd

F32 = mybir.dt.float32

B, T_FULL, I, H = 512, 2048, 13, 50
NCORES = 8
BS = B // NCORES          # 64 sequences per core
SB = 8                    # timesteps per block (one PSUM bank of 512 cols)
NF = SB * BS              # 512 free elements per block
KX = I + 1                # 14: input features + mask row
KILL = -60.0              # o-gate pre-activation offset for dead steps
Q = 64                    # partition offset of the second gate in a bank
P2 = Q + H                # 114


def _build_graph(nc: bass.Bass, t_steps: int):
    nblk = t_steps // SB

    xc = nc.dram_tensor("xc", [nblk, KX, NF], F32, kind="ExternalInput")
    # x-projection weights, [KX, H] blocks per gate (i, f, g, o) side by side
    wx = nc.dram_tensor("wx", [KX, 4 * Q], F32, kind="ExternalInput")
    # h-projection weights, [H, Q] blocks per gate side by side (zero-padded)
    wh = nc.dram_tensor("wh", [H, 4 * Q], F32, kind="ExternalInput")
    wy = nc.dram_tensor("wy", [H, I], F32, kind="ExternalInput")
    bab = nc.dram_tensor("bab", [128, 1], F32, kind="ExternalInput")  # i,f bias padded
    bg = nc.dram_tensor("bg", [H, 1], F32, kind="ExternalInput")
    bo = nc.dram_tensor("bo", [H, 1], F32, kind="ExternalInput")
    bout = nc.dram_tensor("bout", [I, 1], F32, kind="ExternalInput")
    out = nc.dram_tensor("out", [nblk, I, NF], F32, kind="ExternalOutput")

    Sig = mybir.ActivationFunctionType.Sigmoid
    Tanh = mybir.ActivationFunctionType.Tanh

    with tile.TileContext(nc) as tc:
        with (
            tc.tile_pool(name="consts", bufs=1) as cpool,
            tc.tile_pool(name="state", bufs=1) as spool,
            tc.tile_pool(name="hist", bufs=2) as hpool,
            tc.tile_pool(name="step", bufs=3) as stpool,
            tc.tile_pool(name="yout", bufs=3) as ypool,
            tc.tile_pool(name="psA", bufs=2, space=bass.MemorySpace.PSUM) as psA,
            tc.tile_pool(name="psB", bufs=2, space=bass.MemorySpace.PSUM) as psB,
            tc.tile_pool(name="psY", bufs=2, space=bass.MemorySpace.PSUM) as psY,
            tc.tile_pool(name="psS", bufs=1, space=bass.MemorySpace.PSUM) as psS,
        ):
            # --- constants ---
            wx_t = cpool.tile([128, 4 * Q], F32, tag="wx")
            wh_t = cpool.tile([H, 4 * Q], F32, tag="wh")
            wy_t = cpool.tile([H, I], F32, tag="wy")
            bab_t = cpool.tile([128, 1], F32, tag="bab")
            bg_t = cpool.tile([H, 1], F32, tag="bg")
            bo_t = cpool.tile([H, 1], F32, tag="bo")
            bout_t = cpool.tile([I, 1], F32, tag="bout")
            for _g in range(3):
                nc.sync.dma_start(wx_t[32 * _g : 32 * _g + KX], wx[:])
            nc.sync.dma_start(wh_t[:], wh[:])
            nc.sync.dma_start(wy_t[:], wy[:])
            nc.sync.dma_start(bab_t[:], bab[:])
            nc.sync.dma_start(bg_t[:], bg[:])
            nc.sync.dma_start(bo_t[:], bo[:])
            nc.sync.dma_start(bout_t[:], bout[:])

            def wxg(g, grp):
                return wx_t[32 * grp : 32 * grp + KX, g * Q : (g + 1) * Q]

            def whg(g):
                return wh_t[:, g * Q : (g + 1) * Q]

            # --- whole-x preload: group g lives at partitions 32g..32g+14,
            # covering blocks g*cap..(g+1)*cap ---
            cap = (nblk + 2) // 3
            x_sb = spool.tile([128, cap * NF], F32, tag="xsb")
            for g in range(3):
                g0, g1 = g * cap, min((g + 1) * cap, nblk)
                if g0 >= g1:
                    break
                n_g = g1 - g0
                dst = x_sb[32 * g : 32 * g + KX, 0 : n_g * NF]
                nc.sync.dma_start(
                    dst.rearrange("k (n f) -> k n f", f=NF),
                    xc[g0:g1].rearrange("n k f -> k n f"),
                )

            # --- persistent state ---
            # X rows 0:50 = tanh(g) of current step, rows 64:114 = c
            X = spool.tile([128, BS], F32, tag="X")
            h0 = spool.tile([H, BS], F32, tag="h0")
            nc.vector.memset(X[:], 0.0)
            nc.vector.memset(h0[:], 0.0)

            scr = psS.tile([1, 4], F32, tag="scr")
            prev_h = h0[:]
            for blk in range(nblk):
                xg, xr = blk // cap, blk % cap
                if xg > 0 and xr == 0:
                    # absorb group-g DMA completion into PE's clock so real
                    # matmuls keep <=2 sync waits
                    nc.tensor.matmul(
                        scr[:], x_sb[32 * xg : 32 * xg + 1, 0:1],
                        x_sb[32 * xg : 32 * xg + 1, 0:4],
                        start=True, stop=True, skip_group_check=True,
                    )
                xt = x_sb[32 * xg : 32 * xg + KX, xr * NF : (xr + 1) * NF]

                A = psA.tile([128, NF], F32, tag="A")
                Bp = psB.tile([128, NF], F32, tag="B")
                # xproj: i, f -> A; g, o -> B
                for g, (ps, r0) in enumerate(
                    ((A, 0), (A, Q), (Bp, 0), (Bp, Q))
                ):
                    nc.tensor.matmul(
                        ps[r0 : r0 + Q], wxg(g, xg), xt,
                        start=True, stop=False, skip_group_check=True,
                    )

                Ht = hpool.tile([H, NF], F32, tag="H")
                for s in range(SB):
                    c0, c1 = s * BS, (s + 1) * BS
                    for g, (ps, r0) in enumerate(
                        ((A, 0), (A, Q), (Bp, 0), (Bp, Q))
                    ):
                        nc.tensor.matmul(
                            ps[r0 : r0 + Q, c0:c1], whg(g), prev_h,
                            start=False, stop=True, skip_group_check=True,
                        )
                    S_if = stpool.tile([128, BS], F32, tag="Sif")
                    O = stpool.tile([H, BS], F32, tag="O")
                    U = stpool.tile([H, BS], F32, tag="U")
                    V = stpool.tile([H, BS], F32, tag="V")
                    Th = stpool.tile([H, BS], F32, tag="Th")
                    nc.scalar.activation(
                        S_if[0:P2], A[0:P2, c0:c1], Sig, bias=bab_t[0:P2]
                    )
                    nc.scalar.activation(
                        X[0:H], Bp[0:H, c0:c1], Tanh, bias=bg_t[:]
                    )
                    nc.scalar.activation(
                        O[:], Bp[Q:P2, c0:c1], Sig, bias=bo_t[:]
                    )
                    # i*g (bases 0/0) and f*c (bases 64/64); outputs at base 0
                    nc.vector.tensor_mul(U[:], S_if[0:H], X[0:H])
                    nc.vector.tensor_mul(V[:], S_if[Q:P2], X[Q:P2])
                    # c_new = i*g + f*c (bases 0/0), written back to c @64
                    nc.vector.tensor_add(X[Q:P2], U[:], V[:])
                    nc.scalar.activation(Th[:], X[Q:P2], Tanh)
                    nc.vector.tensor_mul(Ht[:, c0:c1], O[:], Th[:])
                    prev_h = Ht[:, c0:c1]

                Y = psY.tile([I, NF], F32, tag="Y")
                nc.tensor.matmul(Y[:], wy_t[:], Ht[:], start=True, stop=True)
                yt = ypool.tile([I, NF], F32, tag="yt")
                nc.vector.tensor_scalar_add(yt[:], Y[:], bout_t[:])
                nc.sync.dma_start(out[blk], yt[:])

    nc.compile()
    return nc


def _prep_inputs(x, lengths, W_ih, W_hh, b_ih, b_hh, W_out, b_out, t_steps):
    """Build per-core in_maps. Gate row order in torch weights: i,f,g,o."""
    nblk = t_steps // SB
    b_all = (b_ih + b_hh).astype(np.float32)

    wx = np.zeros((KX, 4 * Q), np.float32)
    wh = np.zeros((H, 4 * Q), np.float32)
    for g in range(4):
        wx[:I, g * Q : g * Q + H] = W_ih[g * H : (g + 1) * H].T
        wh[:, g * Q : g * Q + H] = W_hh[g * H : (g + 1) * H].T
    wx[I, 3 * Q : 3 * Q + H] = KILL           # kill o-gate when mask row == 1

    bab = np.zeros((128, 1), np.float32)
    bab[0:H, 0] = b_all[0:H]                  # i
    bab[Q:P2, 0] = b_all[H : 2 * H]           # f

    shared = {
        "wx": wx,
        "wh": wh,
        "wy": np.ascontiguousarray(W_out.T.astype(np.float32)),
        "bab": bab,
        "bg": np.ascontiguousarray(b_all[2 * H : 3 * H][:, None]),
        "bo": np.ascontiguousarray(b_all[3 * H : 4 * H][:, None]),
        "bout": np.ascontiguousarray(b_out.astype(np.float32)[:, None]),
    }

    in_maps = []
    tidx = np.arange(t_steps)[:, None]
    for c in range(NCORES):
        xb = x[c * BS : (c + 1) * BS, :t_steps]          # [BS, T, I]
        lb = lengths[c * BS : (c + 1) * BS]              # [BS]
        xt = xb.transpose(1, 2, 0)                       # [T, I, BS]
        xt = (
            xt.reshape(nblk, SB, I, BS)
            .transpose(0, 2, 1, 3)
            .reshape(nblk, I, NF)
        )
        dead = (tidx >= lb[None, :]).astype(np.float32)  # [T, BS]
        dead = (
            dead.reshape(nblk, SB, 1, BS)
            .transpose(0, 2, 1, 3)
            .reshape(nblk, 1, NF)
        )
        xcn = np.ascontiguousarray(
            np.concatenate([xt, dead], axis=1).astype(np.float32)
        )
        in_maps.append({"xc": xcn, **shared})
    return in_maps


def _postprocess(results, t_steps):
    nblk = t_steps // SB
    y = np.empty((B, t_steps, I), np.float32)
    for c in range(NCORES):
        oc = results[c]["out"]                            # [nblk, I, NF]
        yc = (
            oc.reshape(nblk, I, SB, BS)
            .transpose(3, 0, 2, 1)
            .reshape(BS, t_steps, I)
        )
        y[c * BS : (c + 1) * BS] = yc
    return y


def _enable_axon_trace():
    """Install the NTFF profile hook that this image's antenv lacks, so
    run_bass_kernel_spmd(trace=True) can capture exec_time_ns + perfetto."""
    import contextlib
    import ctypes
    import sys
    import types

    if "antenv.axon_hooks" in sys.modules:
        return
    lib = ctypes.CDLL("/opt/axon/libaxon_pjrt.so")
    lib.axon_start_nrt_profile.argtypes = [
        ctypes.POINTER(ctypes.c_int64), ctypes.c_size_t,
    ]
    lib.axon_start_nrt_profile.restype = ctypes.c_int64
    lib.axon_stop_nrt_profile.argtypes = [ctypes.c_char_p]
    lib.axon_stop_nrt_profile.restype = ctypes.c_int64

    @contextlib.contextmanager
    def _hook(output_dir, device_ids):
        import jax
        jax.devices()
        if device_ids:
            ids = (ctypes.c_int64 * len(device_ids))(*device_ids)
            rc = lib.axon_start_nrt_profile(ids, len(device_ids))
        else:
            rc = lib.axon_start_nrt_profile(None, 0)
        if rc != 0:
            raise RuntimeError(f"axon_start_nrt_profile rc={rc}")
        try:
            yield
        finally:
            n = lib.axon_stop_nrt_profile(str(output_dir).encode())
            print(f"profile: {n} file(s) written to {output_dir}")

    mod = types.ModuleType("antenv.axon_hooks")
    mod.get_axon_ntff_profile_hook = lambda: _hook
    mod.set_axon_ntff_profile_hook = lambda h: None
    sys.modules["antenv.axon_hooks"] = mod

    from concourse import bass_utils as bu
    bu.upload_artifacts = lambda tmpdir: f"local://{tmpdir}"


def _run(x, lengths, W_ih, W_hh, b_ih, b_hh, W_out, b_out, t_steps=T_FULL,
         trace=False, tmpdir=None):
    x = np.asarray(x, np.float32)
    lengths = np.asarray(lengths, np.int32)
    W_ih = np.asarray(W_ih, np.float32)
    W_hh = np.asarray(W_hh, np.float32)
    b_ih = np.asarray(b_ih, np.float32)
    b_hh = np.asarray(b_hh, np.float32)
    W_out = np.asarray(W_out, np.float32)
    b_out = np.asarray(b_out, np.float32)

    nc = bacc.Bacc()
    _build_graph(nc, t_steps)
    in_maps = _prep_inputs(
        x, lengths, W_ih, W_hh, b_ih, b_hh, W_out, b_out, t_steps
    )
    if trace:
        _enable_axon_trace()
    res = run_bass_kernel_spmd(
        nc, in_maps, core_ids=list(range(NCORES)), trace=trace, tmpdir=tmpdir
    )
    y = _postprocess(res.results, t_steps)
    return y, res.exec_time_ns


def kernel(x, lengths, W_ih, W_hh, b_ih, b_hh, W_out, b_out):
    y, _ = _run(x, lengths, W_ih, W_hh, b_ih, b_hh, W_out, b_out)
    return y


def kernel_with_time(x, lengths, W_ih, W_hh, b_ih, b_hh, W_out, b_out,
                     t_steps=T_FULL, trace=True, tmpdir=None):
    return _run(x, lengths, W_ih, W_hh, b_ih, b_hh, W_out, b_out,
                t_steps=t_steps, trace=trace, tmpdir=tmpdir)
